# revision 5
# baseline (speedup 1.0000x reference)
"""CrossFeatureAttention TRN2 kernel.

Full inputs -> full output. Sharding: data-parallel over (batch b, half of N1)
across 8 cores; each core computes out[b, h*2048:(h+1)*2048, :].

Math (per core, x1 slice q=2048 rows, x2[b] k=4096 rows, C=512):
    Q  = x1 @ Wq^T + bq
    K  = x2 @ Wk^T + bk
    V  = x2 @ Wv^T + bv
    P  = softmax(Q K^T / sqrt(C))          (no max subtraction; scores are small)
    out = (Q + P V) @ Wo^T + bo
        = x1 @ (Wo Wq)^T + (P V) @ Wo^T + (Wo bq + bo)     <- residual folded

The x1 @ (Wo Wq)^T term carries almost all of the output magnitude and runs in
fp32r; the attention path runs in bf16.  Attention is computed transposed
(S^T[k,q] = sum_c K^T[c,k] Q^T[c,q]) so exp(S^T) is already in the layout the
A^T matmul needs, and row sums come from a ones-matmul over partitions.
"""

import os
import sys

import numpy as np

for _p in ("/root/.axon_site", "/root/.axon_site/_ro/trn_rl_repo",
           "/root/.axon_site/_ro/pypackages"):
    if _p not in sys.path and os.path.isdir(_p):
        sys.path.append(_p)

import ml_dtypes

import concourse.bacc as bacc
import concourse.mybir as mybir
import concourse.tile as tile
from concourse import bass_isa, library_config, masks
from concourse.bass_utils import run_bass_kernel_spmd

F32 = mybir.dt.float32
F32R = mybir.dt.float32r
BF16 = mybir.dt.bfloat16
AF = mybir.ActivationFunctionType

B, N1, N2, C = 4, 4096, 4096, 512
NCORES = 8
QROWS = N1 * B // NCORES          # 2048 q rows per core
QC = 512                          # q-chunk (columns of S^T tiles)
NQC = QROWS // QC                 # 4 chunks
KT = N2 // 128                    # 32 k-tiles
CCH = C // 128                    # 4 contraction chunks
SCALE = 1.0 / float(np.sqrt(C))

_BUILT = None


def build():
    nc = bacc.Bacc(None, target_bir_lowering=False, debug=False)

    x1f_d = nc.dram_tensor("x1f", [QROWS, C], F32, kind="ExternalInput")
    x1b_d = nc.dram_tensor("x1b", [QROWS, C], BF16, kind="ExternalInput")
    x2b_d = nc.dram_tensor("x2b", [N2, C], BF16, kind="ExternalInput")
    wq_d = nc.dram_tensor("wq_t", [C, C], BF16, kind="ExternalInput")
    wk_d = nc.dram_tensor("wk_t", [C, C], BF16, kind="ExternalInput")
    wv_d = nc.dram_tensor("wv_t", [C, C], BF16, kind="ExternalInput")
    wo_d = nc.dram_tensor("wo_t", [C, C], BF16, kind="ExternalInput")
    wqo_d = nc.dram_tensor("wqo_t", [C, C], F32, kind="ExternalInput")
    bq_d = nc.dram_tensor("bq", [C], F32, kind="ExternalInput")
    bk_d = nc.dram_tensor("bk", [C], F32, kind="ExternalInput")
    bv_d = nc.dram_tensor("bv", [C], F32, kind="ExternalInput")
    bo2_d = nc.dram_tensor("bo2", [C], F32, kind="ExternalInput")
    out_d = nc.dram_tensor("out", [QROWS, C], F32, kind="ExternalOutput")

    with tile.TileContext(nc) as tc:
        with tc.tile_pool(name="cst", bufs=1) as cst, \
             tc.tile_pool(name="per", bufs=1) as per, \
             tc.tile_pool(name="sb", bufs=1) as sb, \
             tc.tile_pool(name="ps", bufs=1, space="PSUM") as ps:

            # ---- constants / weights ----
            ident = cst.tile([128, 128], F32)
            masks.make_identity(nc, ident[:])
            nc.gpsimd.load_library(library_config.attn)

            def load_w_bf(dram, nm):
                ts = []
                for cc in range(CCH):
                    t = cst.tile([128, C], BF16, name=f"{nm}{cc}", tag=f"{nm}{cc}")
                    nc.gpsimd.dma_start(out=t[:], in_=dram[cc * 128:(cc + 1) * 128, :])
                    ts.append(t)
                return ts

            wk_t = load_w_bf(wk_d, "wk")
            wv_t = load_w_bf(wv_d, "wv")

            bk_t = []
            for d in range(CCH):
                t2 = cst.tile([128, 1], F32, name=f"bk{d}", tag=f"bk{d}")
                nc.gpsimd.dma_start(out=t2[:], in_=bk_d[d * 128:(d + 1) * 128].unsqueeze(1))
                bk_t.append(t2)
            bv_bc = cst.tile([128, C], F32)
            nc.gpsimd.dma_start(out=bv_bc[:], in_=bv_d[:].unsqueeze(0).broadcast_to([128, C]))

            # ---- persistent tensors ----
            kt_b = [per.tile([128, N2], BF16, name=f"ktb{cc}", tag=f"ktb{cc}")
                    for cc in range(CCH)]
            v_b = [per.tile([128, C], BF16, name=f"vb{i}", tag=f"vb{i}")
                   for i in range(KT)]

            # ---- phase X2: K^T and V ----
            for kc0 in range(N2 // 512):
                x2bt = []
                for cc in range(CCH):
                    t = sb.tile([128, 512], BF16, name=f"x2bt{cc}", tag=f"x2bt{cc}", bufs=3)
                    eng = nc.sync if cc % 2 == 0 else nc.scalar
                    eng.dma_start_transpose(
                        t[:], x2b_d[kc0 * 512:(kc0 + 1) * 512, cc * 128:(cc + 1) * 128])
                    x2bt.append(t)
                # K^T[d, k-block]
                for d in range(CCH):
                    pp = ps.tile([128, 512], F32, name="kps", tag="pB", bufs=3)
                    for cc in range(CCH):
                        nc.tensor.matmul(pp[:], lhsT=wk_t[cc][:, d * 128:(d + 1) * 128],
                                         rhs=x2bt[cc][:],
                                         start=(cc == 0), stop=(cc == CCH - 1))
                    nc.vector.tensor_add(
                        out=kt_b[d][:, kc0 * 512:(kc0 + 1) * 512],
                        in0=pp[:], in1=bk_t[d][:].broadcast_to([128, 512]))
                # V[k-subtile, :]
                for kb in range(4):
                    pp = ps.tile([128, C], F32, name="vps", tag="pB", bufs=3)
                    for cc in range(CCH):
                        nc.tensor.matmul(pp[:], lhsT=x2bt[cc][:, kb * 128:(kb + 1) * 128],
                                         rhs=wv_t[cc][:],
                                         start=(cc == 0), stop=(cc == CCH - 1))
                    nc.vector.tensor_add(out=v_b[kc0 * 4 + kb][:], in0=pp[:], in1=bv_bc[:])

            # ---- late weights: Q/Wqo/Wo paths (needed from chunk 0 on) ----
            wq_b = load_w_bf(wq_d, "wq")
            wqo_r = []
            for cc in range(CCH):
                stage2 = sb.tile([128, C], F32, name=f"wqos{cc}", tag="x1f1", bufs=2)
                nc.gpsimd.dma_start(out=stage2[:], in_=wqo_d[cc * 128:(cc + 1) * 128, :])
                t2 = cst.tile([128, C], F32R, name=f"wqo{cc}", tag=f"wqo{cc}")
                nc.scalar.copy(t2[:], stage2[:])
                wqo_r.append(t2)
            wo_t = load_w_bf(wo_d, "wo")
            bq_t = []
            for d in range(CCH):
                t1 = cst.tile([128, 1], F32, name=f"bq{d}", tag=f"bq{d}")
                nc.gpsimd.dma_start(out=t1[:], in_=bq_d[d * 128:(d + 1) * 128].unsqueeze(1))
                bq_t.append(t1)
            bo2_bc = cst.tile([128, C], F32)
            nc.gpsimd.dma_start(out=bo2_bc[:], in_=bo2_d[:].unsqueeze(0).broadcast_to([128, C]))

            # ---- per q-chunk: transpose x1, Q^T, S^T/exp, rowsum, A^T, O ----
            for qc in range(NQC):
                q0 = qc * QC
                # x1 fp32 rows in, PE-transpose to x1t (f32r)
                x1f_in = []
                for rb in range(QC // 128):
                    t = sb.tile([128, C], F32, name=f"x1f{rb}", tag=f"x1f{rb}", bufs=2)
                    eng = nc.sync if rb % 2 == 0 else nc.scalar
                    eng.dma_start(out=t[:], in_=x1f_d[q0 + rb * 128:q0 + (rb + 1) * 128, :])
                    x1f_in.append(t)
                x1t_r = [sb.tile([128, QC], F32R, name=f"x1t{cc}", tag=f"x1t{cc}", bufs=2)
                         for cc in range(CCH)]
                for rb in range(QC // 128):
                    for cc in range(CCH):
                        tp = ps.tile([128, 128], F32, name="tps", tag="pA", bufs=3)
                        nc.tensor.transpose(tp[:], x1f_in[rb][:, cc * 128:(cc + 1) * 128],
                                            ident[:])
                        nc.scalar.copy(x1t_r[cc][:, rb * 128:(rb + 1) * 128], tp[:])
                # x1^T bf16 via xbar DMA for the Q projection
                x1bt = []
                for cc in range(CCH):
                    t = sb.tile([128, QC], BF16, name=f"x1bt{cc}", tag=f"x1bt{cc}", bufs=2)
                    eng = nc.sync if cc % 2 == 0 else nc.scalar
                    eng.dma_start_transpose(
                        t[:], x1b_d[q0:q0 + QC, cc * 128:(cc + 1) * 128])
                    x1bt.append(t)
                # Q^T (bf16) [d, q-chunk]
                qt_bf = []
                for d in range(CCH):
                    pp = ps.tile([128, QC], F32, name="qps", tag="pB", bufs=3)
                    for cc in range(CCH):
                        nc.tensor.matmul(pp[:], lhsT=wq_b[cc][:, d * 128:(d + 1) * 128],
                                         rhs=x1bt[cc][:],
                                         start=(cc == 0), stop=(cc == CCH - 1))
                    t = sb.tile([128, QC], BF16, name=f"qt{d}", tag=f"qt{d}", bufs=2)
                    nc.vector.tensor_add(out=t[:], in0=pp[:],
                                         in1=bq_t[d][:].broadcast_to([128, QC]))
                    qt_bf.append(t)
                # S^T tiles + exp -> pt[kt]
                pt = []
                for kt in range(KT):
                    pp = ps.tile([128, QC], F32, name="sps", tag="pA", bufs=3)
                    for cc in range(CCH):
                        nc.tensor.matmul(pp[:], lhsT=kt_b[cc][:, kt * 128:(kt + 1) * 128],
                                         rhs=qt_bf[cc][:],
                                         start=(cc == 0), stop=(cc == CCH - 1))
                    t = sb.tile([128, QC], BF16, name=f"pt{kt}", tag=f"pt{kt}", bufs=1)
                    nc.scalar.activation(t[:], pp[:], AF.Exp, scale=float(SCALE))
                    pt.append(t)
                # rowsum: DVE partial sums over k-tiles, then cross-partition
                # all-reduce on GpSimd, then reciprocal
                acc0 = sb.tile([128, QC], F32, name="acc0", tag="acc0", bufs=1)
                acc1 = sb.tile([128, QC], F32, name="acc1", tag="acc1", bufs=1)
                nc.vector.tensor_copy(acc0[:], pt[0][:])
                nc.vector.tensor_copy(acc1[:], pt[1][:])
                for kt in range(2, KT, 2):
                    nc.vector.tensor_add(out=acc0[:], in0=acc0[:], in1=pt[kt][:])
                    nc.vector.tensor_add(out=acc1[:], in0=acc1[:], in1=pt[kt + 1][:])
                nc.vector.tensor_add(out=acc0[:], in0=acc0[:], in1=acc1[:])
                rsum = sb.tile([128, QC], F32, name="rsum", tag="rsum", bufs=1)
                nc.gpsimd.partition_all_reduce(rsum[:], acc0[:], channels=128,
                                               reduce_op=bass_isa.ReduceOp.add)
                recip = sb.tile([128, QC], F32, name="recip", tag="recip", bufs=1)
                nc.vector.reciprocal(recip[:], rsum[:])
                # A^T [d, q-chunk]
                at_bf = []
                for d in range(CCH):
                    pp = ps.tile([128, QC], F32, name="aps", tag="pB", bufs=3)
                    for kt in range(KT):
                        nc.tensor.matmul(pp[:], lhsT=v_b[kt][:, d * 128:(d + 1) * 128],
                                         rhs=pt[kt][:],
                                         start=(kt == 0), stop=(kt == KT - 1))
                    t = sb.tile([128, QC], BF16, name=f"at{d}", tag=f"at{d}", bufs=2)
                    nc.vector.tensor_mul(out=t[:], in0=pp[:], in1=recip[:])
                    at_bf.append(t)
                # O = x1 @ Wqo^T (f32r) + A @ Wo^T (bf16) + bo2
                for rb in range(QC // 128):
                    pp = ps.tile([128, C], F32, name="ops", tag="pB", bufs=3)
                    for cc in range(CCH):
                        nc.tensor.matmul(pp[:], lhsT=x1t_r[cc][:, rb * 128:(rb + 1) * 128],
                                         rhs=wqo_r[cc][:],
                                         start=(cc == 0), stop=False)
                    for d in range(CCH):
                        nc.tensor.matmul(pp[:], lhsT=at_bf[d][:, rb * 128:(rb + 1) * 128],
                                         rhs=wo_t[d][:],
                                         start=False, stop=(d == CCH - 1))
                    ot = sb.tile([128, C], F32, name="ot", tag="ot", bufs=3)
                    nc.vector.tensor_add(out=ot[:], in0=pp[:], in1=bo2_bc[:])
                    nc.scalar.dma_start(out=out_d[q0 + rb * 128:q0 + (rb + 1) * 128, :],
                                      in_=ot[:])

    nc.compile()
    return nc


def get_built():
    global _BUILT
    if _BUILT is None:
        _BUILT = build()
    return _BUILT


def make_in_maps(x1, x2, Wq, bq, Wk, bk, Wv, bv, Wo, bo):
    bf = ml_dtypes.bfloat16
    wq_t = np.ascontiguousarray(Wq.T).astype(bf)
    wk_t = np.ascontiguousarray(Wk.T).astype(bf)
    wv_t = np.ascontiguousarray(Wv.T).astype(bf)
    wo_t = np.ascontiguousarray(Wo.T).astype(bf)
    wqo_t = np.ascontiguousarray((Wo @ Wq).T).astype(np.float32)
    bo2 = (Wo @ bq + bo).astype(np.float32)
    in_maps = []
    for cid in range(NCORES):
        b, h = cid // 2, cid % 2
        x1s = np.ascontiguousarray(x1[b, h * QROWS:(h + 1) * QROWS, :])
        in_maps.append({
            "x1f": x1s,
            "x1b": x1s.astype(bf),
            "x2b": np.ascontiguousarray(x2[b]).astype(bf),
            "wq_t": wq_t, "wk_t": wk_t, "wv_t": wv_t, "wo_t": wo_t,
            "wqo_t": wqo_t,
            "bq": bq.astype(np.float32), "bk": bk.astype(np.float32),
            "bv": bv.astype(np.float32), "bo2": bo2,
        })
    return in_maps


LAST_RESULT = None


def kernel(x1, x2, Wq, bq, Wk, bk, Wv, bv, Wo, bo):
    global LAST_RESULT
    nc = get_built()
    in_maps = make_in_maps(x1, x2, Wq, bq, Wk, bk, Wv, bv, Wo, bo)
    trace = bool(os.environ.get("KERNEL_TRACE"))
    res = run_bass_kernel_spmd(nc, in_maps, core_ids=list(range(NCORES)), trace=trace)
    LAST_RESULT = res
    out = np.empty((B, N1, C), dtype=np.float32)
    for cid in range(NCORES):
        b, h = cid // 2, cid % 2
        out[b, h * QROWS:(h + 1) * QROWS, :] = res.results[cid]["out"]
    return out
